# revision 1
# baseline (speedup 1.0000x reference)
"""Trainium2 Bass kernel for nn_Dot_Attention (sparse attention softmax).

Computes, for each mention m:
    alpha[m, s] = (queries[m] . values[m, s]) / sqrt(D)
    valid[m, s] = (s < len[m]) & ~(start[m] <= s < end[m])
    out[m, :]   = softmax(where(valid, alpha, -inf))

Sharding: mention dim (axis 0) split evenly across 8 NeuronCores (pure data
parallel). Host does only the tiny index gathers (len/start/end per mention,
derived from idx/lengths) and the final concat; all heavy compute (dot
products, masking, softmax over 1.5 GiB of values) runs on device.

Device strategy (per core: 256 mentions = 2 blocks of 128):
  - values tiles loaded as [128 mentions, SG, 384] (one contiguous 24 KB run
    per partition, 3 MB per DMA) -> mentions on partitions, so alpha lands
    directly in the softmax-friendly [mention, s] layout; no transposes.
  - dot products: VectorE does ONE pass (tensor_mul of PR s-positions
    against a PR-replicated q); ScalarE does the reduction over D via
    activation(Copy, scale=1/sqrt(D), accum_out=alpha column). This keeps
    VectorE at ~1 pass/element (fp32 tensor_tensor is 1x mode) and puts the
    reduce on the otherwise-idle ScalarE. (tensor_tensor_reduce would fuse
    both on DVE but crashes the exec unit on this hardware/runtime.)
  - mask built from an iota row vs per-partition scalars; applied as
    alpha - C*invalid before a single ScalarE Exp that also emits the row
    sums via accum_out; normalize with reciprocal + tensor_scalar (2x mode).
"""

import math

import numpy as np

M, S, D = 2048, 512, 384
NCORES = 8
ML = M // NCORES          # mentions per core
BLK = 128                 # mentions per block (partition dim)
NBLK = ML // BLK
SG = 16                   # s-positions per values DMA tile (3 MB per DMA)
PR = 4                    # s-positions fused per DVE multiply
SCALE = 1.0 / math.sqrt(D)
BIGC = 300.0              # exp(x - BIGC) == 0.0 in fp32 for masked entries

_NC = {}


def _build(rep=1):
    """Build+compile the per-core Bass module. rep>1 unrolls the whole
    computation rep times (used only by test.py for slope-based timing)."""
    if rep in _NC:
        return _NC[rep]

    import concourse.bacc as bacc
    import concourse.tile as tile
    import concourse.mybir as mybir

    F32 = mybir.dt.float32
    Op = mybir.AluOpType

    nc = bacc.Bacc(
        "TRN2", target_bir_lowering=False, debug=False, num_devices=NCORES
    )
    q_ap = nc.dram_tensor("queries", [ML, D], F32, kind="ExternalInput").ap()
    v_ap = nc.dram_tensor("values", [ML, S, D], F32, kind="ExternalInput").ap()
    s3_ap = nc.dram_tensor("scal3", [ML, 3], F32, kind="ExternalInput").ap()
    io_ap = nc.dram_tensor("iota", [BLK, S], F32, kind="ExternalInput").ap()
    out_ap = nc.dram_tensor("out", [ML, S], F32, kind="ExternalOutput").ap()

    with tile.TileContext(nc) as tc:
        with (
            tc.tile_pool(name="pv", bufs=3) as pv,
            tc.tile_pool(name="pq", bufs=2) as pq,
            tc.tile_pool(name="pa", bufs=2) as pa,
            tc.tile_pool(name="ps", bufs=2) as ps,
            tc.tile_pool(name="pc", bufs=1) as pc,
        ):
            iota_t = pc.tile([BLK, S], F32)
            nc.scalar.dma_start(iota_t[:], io_ap)

            for b in [bb for _ in range(rep) for bb in range(NBLK)]:
                m0 = b * BLK
                # q replicated PR times along free dim so one DVE multiply
                # covers PR s-positions (bigger ops amortize DVE overhead)
                q4 = pq.tile([BLK, PR, D], F32, tag="q4")
                for k in range(PR):
                    nc.scalar.dma_start(q4[:, k, :], q_ap[m0 : m0 + BLK, :])
                sc_t = pq.tile([BLK, 3], F32, tag="sc3")
                nc.scalar.dma_start(sc_t[:], s3_ap[m0 : m0 + BLK, :])

                alpha = pa.tile([BLK, S], F32, tag="alpha")
                for g in range(S // SG):
                    v_t = pv.tile([BLK, SG, D], F32, tag="v")
                    # big V loads stay on the SP HWDGE ring only: putting them
                    # on the ACT ring stalls the activation stream (+40% e2e)
                    nc.sync.dma_start(
                        v_t[:], v_ap[m0 : m0 + BLK, g * SG : (g + 1) * SG, :]
                    )
                    for j0 in range(0, SG, PR):
                        # DVE: one-pass elementwise product for PR s-positions
                        prod = ps.tile([BLK, PR, D], F32, tag="prod", bufs=3)
                        nc.vector.tensor_mul(
                            prod[:], v_t[:, j0 : j0 + PR, :], q4[:]
                        )
                        for k in range(PR):
                            s_idx = g * SG + j0 + k
                            # ACT: reduce over D via activation accum_out
                            # (also applies the 1/sqrt(D) scale)
                            dump = ps.tile([BLK, D], F32, tag="dump", bufs=2)
                            nc.scalar.activation(
                                dump[:],
                                prod[:, k, :],
                                mybir.ActivationFunctionType.Copy,
                                bias=0.0,
                                scale=SCALE,
                                accum_out=alpha[:, s_idx : s_idx + 1],
                            )

                # invalid = (iota >= len) | ((iota >= start) & (iota < end))
                mA = ps.tile([BLK, S], F32, tag="mA")
                nc.vector.tensor_scalar(mA[:], iota_t[:], sc_t[:, 0:1], None, Op.is_ge)
                mB = ps.tile([BLK, S], F32, tag="mB")
                nc.vector.tensor_scalar(mB[:], iota_t[:], sc_t[:, 1:2], None, Op.is_ge)
                msp = ps.tile([BLK, S], F32, tag="msp")
                nc.vector.scalar_tensor_tensor(
                    msp[:], iota_t[:], sc_t[:, 2:3], mB[:], op0=Op.is_lt, op1=Op.mult
                )
                inval = ps.tile([BLK, S], F32, tag="inval")
                nc.vector.tensor_tensor(inval[:], mA[:], msp[:], Op.max)

                # am = alpha - BIGC * invalid ; out = exp(am), sums over s
                am = ps.tile([BLK, S], F32, tag="am")
                nc.vector.scalar_tensor_tensor(
                    am[:], inval[:], -BIGC, alpha[:], op0=Op.mult, op1=Op.add
                )
                expv = pa.tile([BLK, S], F32, tag="expv")
                sums = ps.tile([BLK, 1], F32, tag="sums")
                nc.scalar.activation(
                    expv[:],
                    am[:],
                    mybir.ActivationFunctionType.Exp,
                    bias=0.0,
                    scale=1.0,
                    accum_out=sums[:],
                )
                recip = ps.tile([BLK, 1], F32, tag="recip")
                nc.vector.reciprocal(recip[:], sums[:])
                outt = pa.tile([BLK, S], F32, tag="outt")
                nc.vector.tensor_scalar(outt[:], expv[:], recip[:], None, Op.mult)
                nc.scalar.dma_start(out_ap[m0 : m0 + BLK, :], outt[:])

    nc.compile()
    _NC[rep] = nc
    return nc


def _host_prep(idx, lengths):
    idx = np.asarray(idx)
    lengths = np.asarray(lengths)
    sent = idx[:, 4].astype(np.int64)
    prefix = np.concatenate(
        [np.zeros(1, np.int64), np.cumsum(lengths.astype(np.int64))[:-1]]
    )
    mlen = lengths[sent].astype(np.float32)
    start = (idx[:, 2].astype(np.int64) - prefix[sent]).astype(np.float32)
    end = (idx[:, 3].astype(np.int64) - prefix[sent]).astype(np.float32)
    return np.stack([mlen, start, end], axis=1)  # [M, 3] f32


def kernel(queries, values, idx, lengths):
    from concourse.bass_utils import run_bass_kernel_spmd

    queries = np.ascontiguousarray(np.asarray(queries, dtype=np.float32))
    values = np.ascontiguousarray(np.asarray(values, dtype=np.float32))
    scal3 = _host_prep(idx, lengths)
    iota = np.ascontiguousarray(
        np.broadcast_to(np.arange(S, dtype=np.float32), (BLK, S))
    )

    nc = _build()
    in_maps = [
        {
            "queries": queries[c * ML : (c + 1) * ML],
            "values": values[c * ML : (c + 1) * ML],
            "scal3": scal3[c * ML : (c + 1) * ML],
            "iota": iota,
        }
        for c in range(NCORES)
    ]
    res = run_bass_kernel_spmd(nc, in_maps, core_ids=list(range(NCORES)))
    return np.concatenate([res.results[c]["out"] for c in range(NCORES)], axis=0)



# revision 2
# speedup vs baseline: 1.1992x; 1.1992x over previous
"""Trainium2 Bass kernel for nn_Dot_Attention (sparse attention softmax).

Computes, for each mention m:
    alpha[m, s] = (queries[m] . values[m, s]) / sqrt(D)
    valid[m, s] = (s < len[m]) & ~(start[m] <= s < end[m])
    out[m, :]   = softmax(where(valid, alpha, -inf))

Sharding: mention dim (axis 0) split across 8 NeuronCores (data parallel),
with a host-side sort by sentence length: mentions are ordered by descending
len[m] and rank r is assigned to core (r % 8), block (r // 8) // 128,
partition (r // 8) % 128.  Because consecutive ranks go to different cores,
all 8 cores see near-identical length profiles, so ONE compiled module (SPMD)
can bake in data-dependent DMA/compute trimming:

  - positions s >= len[m] are masked out of the softmax, so values[m, s, :]
    for those s never needs to leave HBM.  Per (block, s-group of 16) the
    kernel only DMAs the partition prefix [0:k] that can still have valid
    tokens (k from the sorted length profile, max'd across cores); s-groups
    with k == 0 are skipped entirely.  This cuts HBM traffic ~25% and
    compute ~12% (len ~ U[256,512)).

Device strategy per 128-mention block:
  - V tiles [k, 16, 384] f32 DMA'd on the SP HWDGE ring (24 KB/partition
    contiguous runs); mentions on partitions so alpha lands in softmax
    layout.
  - ACT casts V to fp16 (one big instruction per tile, amortizing the
    352-cycle ACT fixed overhead ~20x vs the old per-position reduce).
  - DVE multiplies V16 by a replicated fp16 q (pre-scaled by 1/sqrt(D)) in
    2x packed-16-bit mode, then does a segmented tensor_reduce(axis=X)
    [k,16,384] -> [k,16] straight into alpha columns.  This replaces the
    1024 per-position ACT reduce instructions (613 ns each; the old
    bottleneck at ~630 us busy) with 64 wide DVE ops.
  - mask from an iota row vs per-partition len/start/end scalars; applied as
    alpha - C*invalid before a single ScalarE Exp that also emits row sums
    via accum_out; normalize with reciprocal + tensor_scalar.

fp16 precision: products q*v are O(1), summed over 384 in fp32; the fp16
rounding adds ~6e-4 abs error to alpha -> ~0.1% on softmax, far inside the
2e-2 gate (measured 4e-4 max rel err).
"""

import math

import numpy as np

M, S, D = 2048, 512, 384
NCORES = 8
ML = M // NCORES          # mentions per core
BLK = 128                 # mentions per block (partition dim)
NBLK = ML // BLK
SG = 16                   # s-positions per values DMA tile (<=3 MB per DMA)
NG = S // SG
SCALE = 1.0 / math.sqrt(D)
BIGC = 300.0              # exp(x - BIGC) == 0.0 in fp32 for masked entries

_NC = {}


def _build(rep=1, ks=None):
    """Build+compile the per-core Bass module. ks[j][g] = partition count to
    process for block j, s-group g (None -> no trimming). rep>1 unrolls the
    whole computation rep times (used only by test.py for slope timing)."""
    if ks is None:
        ks = tuple(tuple(BLK for _ in range(NG)) for _ in range(NBLK))
    key = (rep, ks)
    if key in _NC:
        return _NC[key]

    import concourse.bacc as bacc
    import concourse.tile as tile
    import concourse.mybir as mybir

    F32 = mybir.dt.float32
    F16 = mybir.dt.float16
    Op = mybir.AluOpType
    AF = mybir.ActivationFunctionType

    nc = bacc.Bacc(
        "TRN2", target_bir_lowering=False, debug=False, num_devices=NCORES
    )
    q_ap = nc.dram_tensor("queries", [ML, D], F32, kind="ExternalInput").ap()
    v_ap = nc.dram_tensor("values", [ML, S, D], F32, kind="ExternalInput").ap()
    s3_ap = nc.dram_tensor("scal3", [ML, 3], F32, kind="ExternalInput").ap()
    io_ap = nc.dram_tensor("iota", [BLK, S], F32, kind="ExternalInput").ap()
    out_ap = nc.dram_tensor("out", [ML, S], F32, kind="ExternalOutput").ap()

    with tile.TileContext(nc) as tc:
        with (
            tc.tile_pool(name="pv", bufs=2) as pv,
            tc.tile_pool(name="p16", bufs=3) as p16,
            tc.tile_pool(name="pp", bufs=2) as pp,
            tc.tile_pool(name="pq", bufs=2) as pq,
            tc.tile_pool(name="pa", bufs=2) as pa,
            tc.tile_pool(name="ps", bufs=2) as ps,
            tc.tile_pool(name="pc", bufs=1) as pc,
        ):
            iota_t = pc.tile([BLK, S], F32)
            nc.scalar.dma_start(iota_t[:], io_ap)

            for b in [bb for _ in range(rep) for bb in range(NBLK)]:
                j = b % NBLK
                m0 = j * BLK
                # q: load fp32, cast+scale to fp16, replicate to SG copies so
                # one DVE multiply covers a whole 16-position tile.
                qf = pq.tile([BLK, D], F32, tag="qf")
                nc.scalar.dma_start(qf[:], q_ap[m0 : m0 + BLK, :])
                q16 = pq.tile([BLK, SG, D], F16, tag="q16")
                nc.scalar.activation(
                    q16[:, 0, :], qf[:], AF.Copy, bias=0.0, scale=SCALE
                )
                w = 1
                while w < SG:
                    nc.vector.tensor_scalar(
                        q16[:, w : 2 * w, :], q16[:, 0:w, :], 1.0, None, Op.mult
                    )
                    w *= 2
                sc_t = pq.tile([BLK, 3], F32, tag="sc3")
                nc.scalar.dma_start(sc_t[:], s3_ap[m0 : m0 + BLK, :])

                alpha = pa.tile([BLK, S], F32, tag="alpha")
                nc.vector.memset(alpha[:], 0.0)

                for g in range(NG):
                    k = ks[j][g]
                    if k == 0:
                        continue
                    v_t = pv.tile([BLK, SG, D], F32, tag="v")
                    # big V loads stay on the SP HWDGE ring only: putting them
                    # on the ACT ring stalls the activation stream
                    nc.sync.dma_start(
                        v_t[0:k], v_ap[m0 : m0 + k, g * SG : (g + 1) * SG, :]
                    )
                    v16 = p16.tile([BLK, SG, D], F16, tag="v16")
                    nc.scalar.activation(
                        v16[0:k], v_t[0:k], AF.Copy, bias=0.0, scale=1.0
                    )
                    prod = pp.tile([BLK, SG, D], F16, tag="prod")
                    nc.vector.tensor_mul(prod[0:k], v16[0:k], q16[0:k])
                    nc.vector.tensor_reduce(
                        alpha[0:k, g * SG : (g + 1) * SG],
                        prod[0:k],
                        axis=mybir.AxisListType.X,
                        op=Op.add,
                    )

                # invalid = (iota >= len) | ((iota >= start) & (iota < end))
                mA = ps.tile([BLK, S], F32, tag="mA")
                nc.vector.tensor_scalar(mA[:], iota_t[:], sc_t[:, 0:1], None, Op.is_ge)
                mB = ps.tile([BLK, S], F32, tag="mB")
                nc.vector.tensor_scalar(mB[:], iota_t[:], sc_t[:, 1:2], None, Op.is_ge)
                msp = ps.tile([BLK, S], F32, tag="msp")
                nc.vector.scalar_tensor_tensor(
                    msp[:], iota_t[:], sc_t[:, 2:3], mB[:], op0=Op.is_lt, op1=Op.mult
                )
                inval = ps.tile([BLK, S], F32, tag="inval")
                nc.vector.tensor_tensor(inval[:], mA[:], msp[:], Op.max)

                # am = alpha - BIGC * invalid ; out = exp(am), sums over s
                am = ps.tile([BLK, S], F32, tag="am")
                nc.vector.scalar_tensor_tensor(
                    am[:], inval[:], -BIGC, alpha[:], op0=Op.mult, op1=Op.add
                )
                expv = pa.tile([BLK, S], F32, tag="expv")
                sums = ps.tile([BLK, 1], F32, tag="sums")
                nc.scalar.activation(
                    expv[:],
                    am[:],
                    AF.Exp,
                    bias=0.0,
                    scale=1.0,
                    accum_out=sums[:],
                )
                recip = ps.tile([BLK, 1], F32, tag="recip")
                nc.vector.reciprocal(recip[:], sums[:])
                outt = pa.tile([BLK, S], F32, tag="outt")
                nc.vector.tensor_scalar(outt[:], expv[:], recip[:], None, Op.mult)
                nc.scalar.dma_start(out_ap[m0 : m0 + BLK, :], outt[:])

    nc.compile()
    _NC[key] = nc
    return nc


def _host_prep(idx, lengths):
    idx = np.asarray(idx)
    lengths = np.asarray(lengths)
    sent = idx[:, 4].astype(np.int64)
    prefix = np.concatenate(
        [np.zeros(1, np.int64), np.cumsum(lengths.astype(np.int64))[:-1]]
    )
    mlen = lengths[sent].astype(np.float32)
    start = (idx[:, 2].astype(np.int64) - prefix[sent]).astype(np.float32)
    end = (idx[:, 3].astype(np.int64) - prefix[sent]).astype(np.float32)
    return np.stack([mlen, start, end], axis=1)  # [M, 3] f32


def _plan(mlen):
    """Sort mentions by descending sentence length and derive the shared
    per-(block, s-group) partition counts. Rank r -> core r%8, local row
    r//8. Core 0 holds the longest mention of every 8-rank stripe, so its
    profile upper-bounds all cores and one SPMD module is safe."""
    order = np.argsort(-mlen, kind="stable")
    lead = mlen[order[0::NCORES]]  # [ML], nonincreasing
    ks = tuple(
        tuple(
            int((lead[j * BLK : (j + 1) * BLK] > g * SG).sum()) for g in range(NG)
        )
        for j in range(NBLK)
    )
    return order, ks


def _prep_all(queries, values, idx, lengths):
    queries = np.asarray(queries, dtype=np.float32)
    values = np.asarray(values, dtype=np.float32)
    scal3 = _host_prep(idx, lengths)
    order, ks = _plan(scal3[:, 0])
    iota = np.ascontiguousarray(
        np.broadcast_to(np.arange(S, dtype=np.float32), (BLK, S))
    )
    in_maps = []
    for c in range(NCORES):
        sel = order[c::NCORES]
        in_maps.append(
            {
                "queries": np.ascontiguousarray(queries[sel]),
                "values": np.ascontiguousarray(values[sel]),
                "scal3": np.ascontiguousarray(scal3[sel]),
                "iota": iota,
            }
        )
    return order, ks, in_maps


def kernel(queries, values, idx, lengths):
    from concourse.bass_utils import run_bass_kernel_spmd

    order, ks, in_maps = _prep_all(queries, values, idx, lengths)
    nc = _build(1, ks)
    res = run_bass_kernel_spmd(nc, in_maps, core_ids=list(range(NCORES)))
    out = np.empty((M, S), dtype=np.float32)
    for c in range(NCORES):
        out[order[c::NCORES]] = res.results[c]["out"]
    return out


# revision 6
# speedup vs baseline: 1.3561x; 1.1309x over previous
"""Trainium2 Bass kernel for nn_Dot_Attention (sparse attention softmax).

Computes, for each mention m:
    alpha[m, s] = (queries[m] . values[m, s]) / sqrt(D)
    valid[m, s] = (s < len[m]) & ~(start[m] <= s < end[m])
    out[m, :]   = softmax(where(valid, alpha, -inf))

Sharding: mention dim (axis 0) split across 8 NeuronCores (data parallel),
with a host-side sort by sentence length: mentions are ordered by descending
len[m] and rank r is assigned to core (r % 8), block (r // 8) // 128,
partition (r // 8) % 128.  Because consecutive ranks go to different cores,
all 8 cores see near-identical length profiles, so ONE compiled module (SPMD)
can bake in data-dependent DMA/compute trimming:

  - positions s >= len[m] are masked out of the softmax, so values[m, s, :]
    for those s never needs to leave HBM.  Per (block, s-group of 16) the
    kernel only DMAs the partition prefix [0:k] that can still have valid
    tokens (k from the sorted length profile, max'd across cores); s-groups
    with k == 0 are skipped entirely.  This cuts HBM traffic ~25% and
    compute ~12% (len ~ U[256,512)).

Device strategy per 128-mention block:
  - V tiles [k, 16, 384] f32 DMA'd on the SP HWDGE ring (24 KB/partition
    contiguous runs); mentions on partitions so alpha lands in softmax
    layout.
  - ACT casts V to fp16 (one big instruction per tile, amortizing the
    352-cycle ACT fixed overhead ~20x vs the old per-position reduce).
  - DVE multiplies V16 by a replicated fp16 q (pre-scaled by 1/sqrt(D)) in
    2x packed-16-bit mode, then does a segmented tensor_reduce(axis=X)
    [k,16,384] -> [k,16] straight into alpha columns.  This replaces the
    1024 per-position ACT reduce instructions (613 ns each; the old
    bottleneck at ~630 us busy) with 64 wide DVE ops.
  - mask from an iota row vs per-partition len/start/end scalars; applied as
    alpha - C*invalid before a single ScalarE Exp that also emits row sums
    via accum_out; normalize with reciprocal + tensor_scalar.

fp16 precision: products q*v are O(1), summed over 384 in fp32; the fp16
rounding adds ~6e-4 abs error to alpha -> ~0.1% on softmax, far inside the
2e-2 gate (measured 4e-4 max rel err).
"""

import math

import numpy as np

M, S, D = 2048, 512, 384
NCORES = 8
ML = M // NCORES          # mentions per core
BLK = 128                 # mentions per block (partition dim)
NBLK = ML // BLK
SG = 16                   # s-positions per values DMA tile (<=3 MB per DMA)
NG = S // SG
SCALE = 1.0 / math.sqrt(D)
BIGC = 300.0              # exp(x - BIGC) == 0.0 in fp32 for masked entries
OFF_MOD = 6               # every OFF_MOD-th tile reduces on ACT, not DVE
OFF_PHASE = 2

_NC = {}


def _build(rep=1, ks=None, variant="full"):
    """Build+compile the per-core Bass module. ks[j][g] = partition count to
    process for block j, s-group g (None -> no trimming). rep>1 unrolls the
    whole computation rep times (used only by test.py for slope timing).
    variant: 'full' | 'dma' (V loads only) | 'nodma' (compute only) — the
    reduced variants exist only for timeline-sim bottleneck experiments."""
    if ks is None:
        ks = tuple(tuple(BLK for _ in range(NG)) for _ in range(NBLK))
    key = (rep, ks, variant)
    if key in _NC:
        return _NC[key]

    import concourse.bacc as bacc
    import concourse.tile as tile
    import concourse.mybir as mybir

    F32 = mybir.dt.float32
    F16 = mybir.dt.float16
    Op = mybir.AluOpType
    AF = mybir.ActivationFunctionType

    nc = bacc.Bacc(
        "TRN2", target_bir_lowering=False, debug=False, num_devices=NCORES
    )
    q_ap = nc.dram_tensor("queries", [ML, D], F32, kind="ExternalInput").ap()
    v_ap = nc.dram_tensor("values", [ML, S, D], F32, kind="ExternalInput").ap()
    s3_ap = nc.dram_tensor("scal3", [ML, 3], F32, kind="ExternalInput").ap()
    io_ap = nc.dram_tensor("iota", [BLK, S], F32, kind="ExternalInput").ap()
    out_ap = nc.dram_tensor("out", [ML, S], F32, kind="ExternalOutput").ap()

    with tile.TileContext(nc) as tc:
        with (
            tc.tile_pool(name="pv", bufs=2) as pv,
            tc.tile_pool(name="p16", bufs=3) as p16,
            tc.tile_pool(name="pp", bufs=2) as pp,
            tc.tile_pool(name="pq", bufs=2) as pq,
            tc.tile_pool(name="pa", bufs=2) as pa,
            tc.tile_pool(name="ps", bufs=2) as ps,
            tc.tile_pool(name="pc", bufs=1) as pc,
        ):
            iota_t = pc.tile([BLK, S], F32)
            nc.scalar.dma_start(iota_t[:], io_ap)

            tile_no = 0
            for b in [bb for _ in range(rep) for bb in range(NBLK)]:
                j = b % NBLK
                m0 = j * BLK
                # q: load fp32, cast+scale to fp16; broadcast (stride-0 AP)
                # across the SG positions of each multiply.
                qf = pq.tile([BLK, D], F32, tag="qf")
                nc.scalar.dma_start(qf[:], q_ap[m0 : m0 + BLK, :])
                q16 = pq.tile([BLK, D], F16, tag="q16")
                nc.scalar.activation(
                    q16[:], qf[:], AF.Copy, bias=0.0, scale=SCALE
                )
                sc_t = pq.tile([BLK, 3], F32, tag="sc3")
                nc.scalar.dma_start(sc_t[:], s3_ap[m0 : m0 + BLK, :])

                alpha = pa.tile([BLK, S], F32, tag="alpha")
                nc.vector.memset(alpha[:], 0.0)

                for g in range(NG):
                    k = ks[j][g]
                    if k == 0:
                        continue
                    v_t = pv.tile([BLK, SG, D], F32, tag="v")
                    # big V loads stay on the SP HWDGE ring only: putting them
                    # on the ACT ring stalls the activation stream
                    if variant != "nodma":
                        nc.sync.dma_start(
                            v_t[0:k], v_ap[m0 : m0 + k, g * SG : (g + 1) * SG, :]
                        )
                    if variant == "dma":
                        tile_no += 1
                        continue
                    v16 = p16.tile([BLK, SG, D], F16, tag="v16")
                    nc.scalar.activation(
                        v16[0:k], v_t[0:k], AF.Copy, bias=0.0, scale=1.0
                    )
                    prod = pp.tile([BLK, SG, D], F16, tag="prod")
                    qb = q16[0:k].unsqueeze(1).broadcast_to((k, SG, D))
                    nc.vector.tensor_mul(prod[0:k], v16[0:k], qb)
                    acols = alpha[0:k, g * SG : (g + 1) * SG]
                    if tile_no % OFF_MOD == OFF_PHASE:
                        # reduce on ACT: per-position Copy with accum_out
                        # (keeps DVE below the DMA roofline)
                        dump = pp.tile([BLK, D], F16, tag="dump")
                        for i in range(SG):
                            nc.scalar.activation(
                                dump[0:k],
                                prod[0:k, i, :],
                                AF.Copy,
                                bias=0.0,
                                scale=1.0,
                                accum_out=alpha[0:k, g * SG + i : g * SG + i + 1],
                            )
                    else:
                        # reduce on DVE: 2x-mode in-place halving adds
                        # 384->192->96->48->24, then one 1x reduce of 24
                        with nc.allow_low_precision(
                            reason="fp16 pair-sums; fp32 accum in final reduce"
                        ):
                            w = D // 2
                            while w >= 24:
                                nc.vector.tensor_tensor(
                                    prod[0:k, :, 0:w],
                                    prod[0:k, :, 0:w],
                                    prod[0:k, :, w : 2 * w],
                                    Op.add,
                                )
                                w //= 2
                        nc.vector.tensor_reduce(
                            acols,
                            prod[0:k, :, 0:24],
                            axis=mybir.AxisListType.X,
                            op=Op.add,
                        )
                    tile_no += 1

                # invalid = (iota >= len) | ((iota >= start) & (iota < end))
                mA = ps.tile([BLK, S], F32, tag="mA")
                nc.vector.tensor_scalar(mA[:], iota_t[:], sc_t[:, 0:1], None, Op.is_ge)
                mB = ps.tile([BLK, S], F32, tag="mB")
                nc.vector.tensor_scalar(mB[:], iota_t[:], sc_t[:, 1:2], None, Op.is_ge)
                msp = ps.tile([BLK, S], F32, tag="msp")
                nc.vector.scalar_tensor_tensor(
                    msp[:], iota_t[:], sc_t[:, 2:3], mB[:], op0=Op.is_lt, op1=Op.mult
                )
                inval = ps.tile([BLK, S], F32, tag="inval")
                nc.vector.tensor_tensor(inval[:], mA[:], msp[:], Op.max)

                # am = alpha - BIGC * invalid ; out = exp(am), sums over s
                am = ps.tile([BLK, S], F32, tag="am")
                nc.vector.scalar_tensor_tensor(
                    am[:], inval[:], -BIGC, alpha[:], op0=Op.mult, op1=Op.add
                )
                expv = pa.tile([BLK, S], F32, tag="expv")
                sums = ps.tile([BLK, 1], F32, tag="sums")
                nc.scalar.activation(
                    expv[:],
                    am[:],
                    AF.Exp,
                    bias=0.0,
                    scale=1.0,
                    accum_out=sums[:],
                )
                recip = ps.tile([BLK, 1], F32, tag="recip")
                nc.vector.reciprocal(recip[:], sums[:])
                outt = pa.tile([BLK, S], F32, tag="outt")
                nc.vector.tensor_scalar(outt[:], expv[:], recip[:], None, Op.mult)
                nc.scalar.dma_start(out_ap[m0 : m0 + BLK, :], outt[:])

    nc.compile()
    _NC[key] = nc
    return nc


def _host_prep(idx, lengths):
    idx = np.asarray(idx)
    lengths = np.asarray(lengths)
    sent = idx[:, 4].astype(np.int64)
    prefix = np.concatenate(
        [np.zeros(1, np.int64), np.cumsum(lengths.astype(np.int64))[:-1]]
    )
    mlen = lengths[sent].astype(np.float32)
    start = (idx[:, 2].astype(np.int64) - prefix[sent]).astype(np.float32)
    end = (idx[:, 3].astype(np.int64) - prefix[sent]).astype(np.float32)
    return np.stack([mlen, start, end], axis=1)  # [M, 3] f32


def _plan(mlen):
    """Sort mentions by descending sentence length and derive the shared
    per-(block, s-group) partition counts. Rank r -> core r%8, local row
    r//8. Core 0 holds the longest mention of every 8-rank stripe, so its
    profile upper-bounds all cores and one SPMD module is safe."""
    order = np.argsort(-mlen, kind="stable")
    lead = mlen[order[0::NCORES]]  # [ML], nonincreasing
    ks = tuple(
        tuple(
            int((lead[j * BLK : (j + 1) * BLK] > g * SG).sum()) for g in range(NG)
        )
        for j in range(NBLK)
    )
    return order, ks


def _prep_all(queries, values, idx, lengths):
    queries = np.asarray(queries, dtype=np.float32)
    values = np.asarray(values, dtype=np.float32)
    scal3 = _host_prep(idx, lengths)
    order, ks = _plan(scal3[:, 0])
    iota = np.ascontiguousarray(
        np.broadcast_to(np.arange(S, dtype=np.float32), (BLK, S))
    )
    in_maps = []
    for c in range(NCORES):
        sel = order[c::NCORES]
        in_maps.append(
            {
                "queries": np.ascontiguousarray(queries[sel]),
                "values": np.ascontiguousarray(values[sel]),
                "scal3": np.ascontiguousarray(scal3[sel]),
                "iota": iota,
            }
        )
    return order, ks, in_maps


def kernel(queries, values, idx, lengths):
    from concourse.bass_utils import run_bass_kernel_spmd

    order, ks, in_maps = _prep_all(queries, values, idx, lengths)
    nc = _build(1, ks)
    res = run_bass_kernel_spmd(nc, in_maps, core_ids=list(range(NCORES)))
    out = np.empty((M, S), dtype=np.float32)
    for c in range(NCORES):
        out[order[c::NCORES]] = res.results[c]["out"]
    return out


# revision 14
# speedup vs baseline: 1.4275x; 1.0526x over previous
"""Trainium2 Bass kernel for nn_Dot_Attention (sparse attention softmax).

Computes, for each mention m:
    alpha[m, s] = (queries[m] . values[m, s]) / sqrt(D)
    valid[m, s] = (s < len[m]) & ~(start[m] <= s < end[m])
    out[m, :]   = softmax(where(valid, alpha, -inf))

Sharding: mention dim (axis 0) split across 8 NeuronCores (data parallel),
with a host-side sort by sentence length: mentions are ordered by descending
len[m] and rank r is assigned to core (r % 8), block (r // 8) // 128,
partition (r // 8) % 128.  Because consecutive ranks go to different cores,
all 8 cores see near-identical length profiles, so ONE compiled module (SPMD)
can bake in data-dependent DMA/compute trimming:

  - positions s >= len[m] are masked out of the softmax, so values[m, s, :]
    for those s never needs to leave HBM.  Per (block, s-group of 16) the
    kernel only DMAs the partition prefix [0:k] that can still have valid
    tokens (k from the sorted length profile, max'd across cores); s-groups
    with k == 0 are skipped entirely.  This cuts HBM traffic ~25% and
    compute ~12% (len ~ U[256,512)).

Device strategy per 128-mention block (per-tile: one s-group of 16):
  - V tile [k, 16, 384] f32 DMA'd on the SP HWDGE ring (24 KB/partition
    contiguous runs); mentions on partitions so alpha lands in softmax
    layout.
  - ACT casts the tile to fp16 (one big instruction, amortizing the
    352-cycle ACT fixed overhead; ACT busy ~305 us, was the 630 us
    bottleneck when it did 1024 per-position accum reduces).
  - DVE multiplies V16 by a stride-0-broadcast fp16 q (pre-scaled by
    1/sqrt(D)) in 2x packed-16-bit mode, then reduces over D with in-place
    2x halving adds 384->192->96->48->24 plus one 1x tensor_reduce of the
    last 24 into alpha columns.  (InstTensorReduce/Pool/TTR have no fast
    DVE modes -- a lone segmented reduce runs 1x and puts DVE at ~590 us;
    the tree keeps DVE at ~395 us, just under the ~430 us DMA roofline.)
  - mask from an iota row vs per-partition len/start/end scalars; applied as
    alpha - C*invalid before a single ScalarE Exp that also emits row sums
    via accum_out; normalize with reciprocal + tensor_scalar.

fp16 precision: products q*v are O(1); pair-sums round in fp16, final 24
accumulate in fp32 -> ~1e-3 abs error on alpha -> ~0.3% on softmax, inside
the 2e-2 gate (measured 2.9e-3 max rel err).
"""

import math

import numpy as np

M, S, D = 2048, 512, 384
NCORES = 8
ML = M // NCORES          # mentions per core
BLK = 128                 # mentions per block (partition dim)
NBLK = ML // BLK
SG = 16                   # s-positions per values DMA tile (<=3 MB per DMA)
NG = S // SG
SCALE = 1.0 / math.sqrt(D)
BIGC = 300.0              # exp(x - BIGC) == 0.0 in fp32 for masked entries
OFF_MOD = 6               # every OFF_MOD-th tile reduces on ACT, not DVE
OFF_PHASE = 2

_NC = {}


def _build(
    rep=1,
    ks=None,
    variant="full",
    reduce_mode="tree",
    off_mod=None,
    ring_split=False,
    deep_bufs=False,
):
    """Build+compile the per-core Bass module. ks[j][g] = partition count to
    process for block j, s-group g (None -> no trimming). rep>1 unrolls the
    whole computation rep times (used only by test.py for slope timing).
    variant: 'full' | 'dma' (V loads only) | 'nodma' (compute only) — the
    reduced variants exist only for timeline-sim bottleneck experiments.
    reduce_mode: 'tree' (2x halving adds) | 'plain' (single 1x tensor_reduce).
    off_mod: offload every off_mod-th tile's reduce to ACT (None = never).
    ring_split: alternate V loads between the SP and ACT HWDGE rings.
    deep_bufs: triple-buffer the V and prod tile pools."""
    if ks is None:
        ks = tuple(tuple(BLK for _ in range(NG)) for _ in range(NBLK))
    key = (rep, ks, variant, reduce_mode, off_mod, ring_split, deep_bufs)
    if key in _NC:
        return _NC[key]

    import concourse.bacc as bacc
    import concourse.tile as tile
    import concourse.mybir as mybir

    F32 = mybir.dt.float32
    F16 = mybir.dt.float16
    Op = mybir.AluOpType
    AF = mybir.ActivationFunctionType

    nc = bacc.Bacc(
        "TRN2", target_bir_lowering=False, debug=False, num_devices=NCORES
    )
    q_ap = nc.dram_tensor("queries", [ML, D], F32, kind="ExternalInput").ap()
    v_ap = nc.dram_tensor("values", [ML, S, D], F32, kind="ExternalInput").ap()
    s3_ap = nc.dram_tensor("scal3", [ML, 3], F32, kind="ExternalInput").ap()
    io_ap = nc.dram_tensor("iota", [BLK, S], F32, kind="ExternalInput").ap()
    out_ap = nc.dram_tensor("out", [ML, S], F32, kind="ExternalOutput").ap()

    nb = 3 if deep_bufs else 2
    with tile.TileContext(nc) as tc:
        with (
            tc.tile_pool(name="pv", bufs=nb) as pv,
            tc.tile_pool(name="p16", bufs=3) as p16,
            tc.tile_pool(name="pp", bufs=nb) as pp,
            tc.tile_pool(name="pq", bufs=2) as pq,
            tc.tile_pool(name="pa", bufs=2) as pa,
            tc.tile_pool(name="ps", bufs=2) as ps,
            tc.tile_pool(name="pc", bufs=1) as pc,
        ):
            iota_t = pc.tile([BLK, S], F32)
            nc.scalar.dma_start(iota_t[:], io_ap)

            tile_no = 0
            for b in [bb for _ in range(rep) for bb in range(NBLK)]:
                j = b % NBLK
                m0 = j * BLK
                # q: load fp32, cast+scale to fp16; broadcast (stride-0 AP)
                # across the SG positions of each multiply.
                qf = pq.tile([BLK, D], F32, tag="qf")
                nc.scalar.dma_start(qf[:], q_ap[m0 : m0 + BLK, :])
                q16 = pq.tile([BLK, D], F16, tag="q16")
                nc.scalar.activation(
                    q16[:], qf[:], AF.Copy, bias=0.0, scale=SCALE
                )
                sc_t = pq.tile([BLK, 3], F32, tag="sc3")
                nc.scalar.dma_start(sc_t[:], s3_ap[m0 : m0 + BLK, :])

                alpha = pa.tile([BLK, S], F32, tag="alpha")
                nc.vector.memset(alpha[:], 0.0)

                for g in range(NG):
                    k = ks[j][g]
                    if k == 0:
                        continue
                    v_t = pv.tile([BLK, SG, D], F32, tag="v")
                    # big V loads stay on the SP HWDGE ring only: putting them
                    # on the ACT ring stalls the activation stream
                    if variant != "nodma":
                        eng = (
                            nc.scalar
                            if (ring_split and tile_no % 2 == 1)
                            else nc.sync
                        )
                        eng.dma_start(
                            v_t[0:k], v_ap[m0 : m0 + k, g * SG : (g + 1) * SG, :]
                        )
                    if variant == "dma":
                        tile_no += 1
                        continue
                    v16 = p16.tile([BLK, SG, D], F16, tag="v16")
                    nc.scalar.activation(
                        v16[0:k], v_t[0:k], AF.Copy, bias=0.0, scale=1.0
                    )
                    prod = pp.tile([BLK, SG, D], F16, tag="prod")
                    qb = q16[0:k].unsqueeze(1).broadcast_to((k, SG, D))
                    nc.vector.tensor_mul(prod[0:k], v16[0:k], qb)
                    acols = alpha[0:k, g * SG : (g + 1) * SG]
                    if off_mod is not None and tile_no % off_mod == OFF_PHASE:
                        # reduce on ACT: per-position Copy with accum_out
                        # (keeps DVE below the DMA roofline)
                        dump = pp.tile([BLK, D], F16, tag="dump")
                        for i in range(SG):
                            nc.scalar.activation(
                                dump[0:k],
                                prod[0:k, i, :],
                                AF.Copy,
                                bias=0.0,
                                scale=1.0,
                                accum_out=alpha[0:k, g * SG + i : g * SG + i + 1],
                            )
                    elif reduce_mode == "tree":
                        # reduce on DVE: 2x-mode in-place halving adds
                        # 384->192->96->48->24, then one 1x reduce of 24
                        with nc.allow_low_precision(
                            reason="fp16 pair-sums; fp32 accum in final reduce"
                        ):
                            w = D // 2
                            while w >= 24:
                                nc.vector.tensor_tensor(
                                    prod[0:k, :, 0:w],
                                    prod[0:k, :, 0:w],
                                    prod[0:k, :, w : 2 * w],
                                    Op.add,
                                )
                                w //= 2
                        nc.vector.tensor_reduce(
                            acols,
                            prod[0:k, :, 0:24],
                            axis=mybir.AxisListType.X,
                            op=Op.add,
                        )
                    else:
                        nc.vector.tensor_reduce(
                            acols,
                            prod[0:k],
                            axis=mybir.AxisListType.X,
                            op=Op.add,
                        )
                    tile_no += 1

                # invalid = (iota >= len) | ((iota >= start) & (iota < end))
                mA = ps.tile([BLK, S], F32, tag="mA")
                nc.vector.tensor_scalar(mA[:], iota_t[:], sc_t[:, 0:1], None, Op.is_ge)
                mB = ps.tile([BLK, S], F32, tag="mB")
                nc.vector.tensor_scalar(mB[:], iota_t[:], sc_t[:, 1:2], None, Op.is_ge)
                msp = ps.tile([BLK, S], F32, tag="msp")
                nc.vector.scalar_tensor_tensor(
                    msp[:], iota_t[:], sc_t[:, 2:3], mB[:], op0=Op.is_lt, op1=Op.mult
                )
                inval = ps.tile([BLK, S], F32, tag="inval")
                nc.vector.tensor_tensor(inval[:], mA[:], msp[:], Op.max)

                # am = alpha - BIGC * invalid ; out = exp(am), sums over s
                am = ps.tile([BLK, S], F32, tag="am")
                nc.vector.scalar_tensor_tensor(
                    am[:], inval[:], -BIGC, alpha[:], op0=Op.mult, op1=Op.add
                )
                expv = pa.tile([BLK, S], F32, tag="expv")
                sums = ps.tile([BLK, 1], F32, tag="sums")
                nc.scalar.activation(
                    expv[:],
                    am[:],
                    AF.Exp,
                    bias=0.0,
                    scale=1.0,
                    accum_out=sums[:],
                )
                recip = ps.tile([BLK, 1], F32, tag="recip")
                nc.vector.reciprocal(recip[:], sums[:])
                outt = pa.tile([BLK, S], F32, tag="outt")
                nc.vector.tensor_scalar(outt[:], expv[:], recip[:], None, Op.mult)
                nc.scalar.dma_start(out_ap[m0 : m0 + BLK, :], outt[:])

    nc.compile()
    _NC[key] = nc
    return nc


def _host_prep(idx, lengths):
    idx = np.asarray(idx)
    lengths = np.asarray(lengths)
    sent = idx[:, 4].astype(np.int64)
    prefix = np.concatenate(
        [np.zeros(1, np.int64), np.cumsum(lengths.astype(np.int64))[:-1]]
    )
    mlen = lengths[sent].astype(np.float32)
    start = (idx[:, 2].astype(np.int64) - prefix[sent]).astype(np.float32)
    end = (idx[:, 3].astype(np.int64) - prefix[sent]).astype(np.float32)
    return np.stack([mlen, start, end], axis=1)  # [M, 3] f32


def _plan(mlen):
    """Sort mentions by descending sentence length and derive the shared
    per-(block, s-group) partition counts. Rank r -> core r%8, local row
    r//8. Core 0 holds the longest mention of every 8-rank stripe, so its
    profile upper-bounds all cores and one SPMD module is safe."""
    order = np.argsort(-mlen, kind="stable")
    lead = mlen[order[0::NCORES]]  # [ML], nonincreasing
    ks = tuple(
        tuple(
            int((lead[j * BLK : (j + 1) * BLK] > g * SG).sum()) for g in range(NG)
        )
        for j in range(NBLK)
    )
    return order, ks


def _prep_all(queries, values, idx, lengths):
    queries = np.asarray(queries, dtype=np.float32)
    values = np.asarray(values, dtype=np.float32)
    scal3 = _host_prep(idx, lengths)
    order, ks = _plan(scal3[:, 0])
    iota = np.ascontiguousarray(
        np.broadcast_to(np.arange(S, dtype=np.float32), (BLK, S))
    )
    in_maps = []
    for c in range(NCORES):
        sel = order[c::NCORES]
        in_maps.append(
            {
                "queries": np.ascontiguousarray(queries[sel]),
                "values": np.ascontiguousarray(values[sel]),
                "scal3": np.ascontiguousarray(scal3[sel]),
                "iota": iota,
            }
        )
    return order, ks, in_maps


def kernel(queries, values, idx, lengths):
    from concourse.bass_utils import run_bass_kernel_spmd

    order, ks, in_maps = _prep_all(queries, values, idx, lengths)
    nc = _build(1, ks)
    res = run_bass_kernel_spmd(nc, in_maps, core_ids=list(range(NCORES)))
    out = np.empty((M, S), dtype=np.float32)
    for c in range(NCORES):
        out[order[c::NCORES]] = res.results[c]["out"]
    return out


# revision 21
# speedup vs baseline: 1.7038x; 1.1936x over previous
"""Trainium2 Bass kernel for nn_Dot_Attention (sparse attention softmax).

Computes, for each mention m:
    alpha[m, s] = (queries[m] . values[m, s]) / sqrt(D)
    valid[m, s] = (s < len[m]) & ~(start[m] <= s < end[m])
    out[m, :]   = softmax(where(valid, alpha, -inf))

Sharding: mention dim (axis 0) split across 8 NeuronCores (data parallel),
with a host-side sort by sentence length: mentions are ordered by descending
len[m] and rank r is assigned to core (r % 8), block (r // 8) // 128,
partition (r // 8) % 128.  Because consecutive ranks go to different cores,
all 8 cores see near-identical length profiles, so ONE compiled module (SPMD)
can bake in data-dependent DMA/compute trimming:

  - positions s >= len[m] are masked out of the softmax, so values[m, s, :]
    for those s never needs to leave HBM.  Per (block, s-group of 16) the
    kernel only DMAs the partition prefix [0:k] that can still have valid
    tokens (k from the sorted length profile, max'd across cores); s-groups
    with k == 0 are skipped entirely.  This cuts HBM traffic ~25% and
    compute ~12% (len ~ U[256,512)).

Device strategy per 128-mention block (per-tile: one s-group of 16):
  - V tile [k, 16, 384] f32 DMA'd on the SP HWDGE ring (24 KB/partition
    contiguous runs); mentions on partitions so alpha lands in softmax
    layout.
  - ACT casts the tile to fp16 (one big instruction, amortizing the
    352-cycle ACT fixed overhead; ACT busy ~305 us, was the 630 us
    bottleneck when it did 1024 per-position accum reduces).
  - DVE multiplies V16 by a stride-0-broadcast fp16 q (pre-scaled by
    1/sqrt(D)) in 2x packed-16-bit mode, then reduces over D with in-place
    2x halving adds 384->192->96->48 plus one 1x tensor_reduce of the last
    48 into alpha columns.  (InstTensorReduce/Pool/TTR have no fast DVE
    modes -- a lone segmented reduce runs 1x and puts DVE at ~590 us; the
    tree keeps DVE at ~395 us, just under the ~430 us DMA roofline.)
    Consecutive s-group pairs share one cast tile and one mul/tree/reduce
    chain (32 positions per instruction) to halve per-instruction fixed
    costs; the k2..k1 partition gap of the second group only ever holds
    memset/stale *finite* fp16s, and those rows are masked to exp->0.
  - mask from an iota row vs per-partition len/start/end scalars; applied as
    alpha - C*invalid before a single ScalarE Exp that also emits row sums
    via accum_out; normalize with reciprocal + tensor_scalar.

fp16 precision: products q*v are O(1); pair-sums round in fp16, final 24
accumulate in fp32 -> ~1e-3 abs error on alpha -> ~0.3% on softmax, inside
the 2e-2 gate (measured 2.9e-3 max rel err).
"""

import math

import numpy as np

M, S, D = 2048, 512, 384
NCORES = 8
ML = M // NCORES          # mentions per core
BLK = 128                 # mentions per block (partition dim)
NBLK = ML // BLK
SG = 16                   # s-positions per values DMA tile (<=3 MB per DMA)
NG = S // SG
SCALE = 1.0 / math.sqrt(D)
BIGC = 300.0              # exp(x - BIGC) == 0.0 in fp32 for masked entries
OFF_MOD = 6               # every OFF_MOD-th tile reduces on ACT, not DVE
OFF_PHASE = 2

_NC = {}


def _build(
    rep=1,
    ks=None,
    variant="full",
    reduce_mode="tree",
    off_mod=None,
    ring_split=False,
    deep_bufs=False,
    pair=True,
    tree_stop=48,
):
    """Build+compile the per-core Bass module. ks[j][g] = partition count to
    process for block j, s-group g (None -> no trimming). rep>1 unrolls the
    whole computation rep times (used only by test.py for slope timing).
    variant: 'full' | 'dma' (V loads only) | 'nodma' (compute only) — the
    reduced variants exist only for timeline-sim bottleneck experiments.
    reduce_mode: 'tree' (2x halving adds) | 'plain' (single 1x tensor_reduce).
    off_mod: offload every off_mod-th tile's reduce to ACT (None = never).
    ring_split: alternate V loads between the SP and ACT HWDGE rings.
    deep_bufs: triple-buffer the V and prod tile pools."""
    if ks is None:
        ks = tuple(tuple(BLK for _ in range(NG)) for _ in range(NBLK))
    key = (rep, ks, variant, reduce_mode, off_mod, ring_split, deep_bufs, pair,
           tree_stop)
    if key in _NC:
        return _NC[key]

    import concourse.bacc as bacc
    import concourse.tile as tile
    import concourse.mybir as mybir

    F32 = mybir.dt.float32
    F16 = mybir.dt.float16
    Op = mybir.AluOpType
    AF = mybir.ActivationFunctionType

    nc = bacc.Bacc(
        "TRN2", target_bir_lowering=False, debug=False, num_devices=NCORES
    )
    q_ap = nc.dram_tensor("queries", [ML, D], F32, kind="ExternalInput").ap()
    v_ap = nc.dram_tensor("values", [ML, S, D], F32, kind="ExternalInput").ap()
    s3_ap = nc.dram_tensor("scal3", [ML, 3], F32, kind="ExternalInput").ap()
    io_ap = nc.dram_tensor("iota", [BLK, S], F32, kind="ExternalInput").ap()
    out_ap = nc.dram_tensor("out", [ML, S], F32, kind="ExternalOutput").ap()

    nb = 3 if deep_bufs else 2
    with tile.TileContext(nc) as tc:
        with (
            tc.tile_pool(name="pv", bufs=nb) as pv,
            tc.tile_pool(name="p16", bufs=3) as p16,
            tc.tile_pool(name="pp", bufs=nb) as pp,
            tc.tile_pool(name="pq", bufs=2) as pq,
            tc.tile_pool(name="pa", bufs=2) as pa,
            tc.tile_pool(name="ps", bufs=2) as ps,
            tc.tile_pool(name="pc", bufs=1) as pc,
        ):
            iota_t = pc.tile([BLK, S], F32)
            nc.scalar.dma_start(iota_t[:], io_ap)

            tile_no = 0
            for b in [bb for _ in range(rep) for bb in range(NBLK)]:
                j = b % NBLK
                m0 = j * BLK
                # q: load fp32, cast+scale to fp16; broadcast (stride-0 AP)
                # across the SG positions of each multiply.
                qf = pq.tile([BLK, D], F32, tag="qf")
                nc.scalar.dma_start(qf[:], q_ap[m0 : m0 + BLK, :])
                q16 = pq.tile([BLK, D], F16, tag="q16")
                nc.scalar.activation(
                    q16[:], qf[:], AF.Copy, bias=0.0, scale=SCALE
                )
                sc_t = pq.tile([BLK, 3], F32, tag="sc3")
                nc.scalar.dma_start(sc_t[:], s3_ap[m0 : m0 + BLK, :])

                alpha = pa.tile([BLK, S], F32, tag="alpha")
                nc.vector.memset(alpha[:], 0.0)

                g = 0
                while pair and variant == "full" and g < NG:
                    k1 = ks[j][g]
                    if k1 == 0:
                        break
                    k2 = ks[j][g + 1] if g + 1 < NG else 0
                    if k2 == 0:
                        break  # odd tail handled by the single-group loop
                    v_t1 = pv.tile([BLK, SG, D], F32, tag="v")
                    nc.sync.dma_start(
                        v_t1[0:k1], v_ap[m0 : m0 + k1, g * SG : (g + 1) * SG, :]
                    )
                    v_t2 = pv.tile([BLK, SG, D], F32, tag="v")
                    nc.sync.dma_start(
                        v_t2[0:k2],
                        v_ap[m0 : m0 + k2, (g + 1) * SG : (g + 2) * SG, :],
                    )
                    v16p = p16.tile([BLK, 2, SG, D], F16, tag="v16p")
                    if tile_no < 6:
                        # first visits of each pool buffer: clear so the
                        # k2..k1 partition gap never feeds NaN bits downstream
                        nc.vector.memset(v16p[:], 0.0)
                    nc.scalar.activation(
                        v16p[0:k1, 0], v_t1[0:k1], AF.Copy, bias=0.0, scale=1.0
                    )
                    nc.scalar.activation(
                        v16p[0:k2, 1], v_t2[0:k2], AF.Copy, bias=0.0, scale=1.0
                    )
                    prodp = pp.tile([BLK, 2, SG, D], F16, tag="prodp")
                    qbp = (
                        q16[0:k1]
                        .unsqueeze(1)
                        .unsqueeze(1)
                        .broadcast_to((k1, 2, SG, D))
                    )
                    nc.vector.tensor_mul(prodp[0:k1], v16p[0:k1], qbp)
                    with nc.allow_low_precision(
                        reason="fp16 pair-sums; fp32 accum in final reduce"
                    ):
                        w = D // 2
                        while w >= tree_stop:
                            nc.vector.tensor_tensor(
                                prodp[0:k1, :, :, 0:w],
                                prodp[0:k1, :, :, 0:w],
                                prodp[0:k1, :, :, w : 2 * w],
                                Op.add,
                            )
                            w //= 2
                    nc.vector.tensor_reduce(
                        alpha[0:k1, g * SG : (g + 2) * SG],
                        prodp[0:k1, :, :, 0:tree_stop],
                        axis=mybir.AxisListType.X,
                        op=Op.add,
                    )
                    tile_no += 2
                    g += 2

                for g in range(g, NG):
                    k = ks[j][g]
                    if k == 0:
                        continue
                    v_t = pv.tile([BLK, SG, D], F32, tag="v")
                    # big V loads stay on the SP HWDGE ring only: putting them
                    # on the ACT ring stalls the activation stream
                    if variant != "nodma":
                        eng = (
                            nc.scalar
                            if (ring_split and tile_no % 2 == 1)
                            else nc.sync
                        )
                        eng.dma_start(
                            v_t[0:k], v_ap[m0 : m0 + k, g * SG : (g + 1) * SG, :]
                        )
                    if variant == "dma":
                        tile_no += 1
                        continue
                    v16 = p16.tile([BLK, SG, D], F16, tag="v16")
                    nc.scalar.activation(
                        v16[0:k], v_t[0:k], AF.Copy, bias=0.0, scale=1.0
                    )
                    prod = pp.tile([BLK, SG, D], F16, tag="prod")
                    qb = q16[0:k].unsqueeze(1).broadcast_to((k, SG, D))
                    nc.vector.tensor_mul(prod[0:k], v16[0:k], qb)
                    acols = alpha[0:k, g * SG : (g + 1) * SG]
                    if off_mod is not None and tile_no % off_mod == OFF_PHASE:
                        # reduce on ACT: per-position Copy with accum_out
                        # (keeps DVE below the DMA roofline)
                        dump = pp.tile([BLK, D], F16, tag="dump")
                        for i in range(SG):
                            nc.scalar.activation(
                                dump[0:k],
                                prod[0:k, i, :],
                                AF.Copy,
                                bias=0.0,
                                scale=1.0,
                                accum_out=alpha[0:k, g * SG + i : g * SG + i + 1],
                            )
                    elif reduce_mode == "tree":
                        # reduce on DVE: 2x-mode in-place halving adds
                        # 384->...->tree_stop, then one 1x reduce
                        with nc.allow_low_precision(
                            reason="fp16 pair-sums; fp32 accum in final reduce"
                        ):
                            w = D // 2
                            while w >= tree_stop:
                                nc.vector.tensor_tensor(
                                    prod[0:k, :, 0:w],
                                    prod[0:k, :, 0:w],
                                    prod[0:k, :, w : 2 * w],
                                    Op.add,
                                )
                                w //= 2
                        nc.vector.tensor_reduce(
                            acols,
                            prod[0:k, :, 0:tree_stop],
                            axis=mybir.AxisListType.X,
                            op=Op.add,
                        )
                    else:
                        nc.vector.tensor_reduce(
                            acols,
                            prod[0:k],
                            axis=mybir.AxisListType.X,
                            op=Op.add,
                        )
                    tile_no += 1

                # invalid = (iota >= len) | ((iota >= start) & (iota < end))
                mA = ps.tile([BLK, S], F32, tag="mA")
                nc.vector.tensor_scalar(mA[:], iota_t[:], sc_t[:, 0:1], None, Op.is_ge)
                mB = ps.tile([BLK, S], F32, tag="mB")
                nc.vector.tensor_scalar(mB[:], iota_t[:], sc_t[:, 1:2], None, Op.is_ge)
                msp = ps.tile([BLK, S], F32, tag="msp")
                nc.vector.scalar_tensor_tensor(
                    msp[:], iota_t[:], sc_t[:, 2:3], mB[:], op0=Op.is_lt, op1=Op.mult
                )
                inval = ps.tile([BLK, S], F32, tag="inval")
                nc.vector.tensor_tensor(inval[:], mA[:], msp[:], Op.max)

                # am = alpha - BIGC * invalid ; out = exp(am), sums over s
                am = ps.tile([BLK, S], F32, tag="am")
                nc.vector.scalar_tensor_tensor(
                    am[:], inval[:], -BIGC, alpha[:], op0=Op.mult, op1=Op.add
                )
                expv = pa.tile([BLK, S], F32, tag="expv")
                sums = ps.tile([BLK, 1], F32, tag="sums")
                nc.scalar.activation(
                    expv[:],
                    am[:],
                    AF.Exp,
                    bias=0.0,
                    scale=1.0,
                    accum_out=sums[:],
                )
                recip = ps.tile([BLK, 1], F32, tag="recip")
                nc.vector.reciprocal(recip[:], sums[:])
                outt = pa.tile([BLK, S], F32, tag="outt")
                nc.vector.tensor_scalar(outt[:], expv[:], recip[:], None, Op.mult)
                nc.scalar.dma_start(out_ap[m0 : m0 + BLK, :], outt[:])

    nc.compile()
    _NC[key] = nc
    return nc


def _host_prep(idx, lengths):
    idx = np.asarray(idx)
    lengths = np.asarray(lengths)
    sent = idx[:, 4].astype(np.int64)
    prefix = np.concatenate(
        [np.zeros(1, np.int64), np.cumsum(lengths.astype(np.int64))[:-1]]
    )
    mlen = lengths[sent].astype(np.float32)
    start = (idx[:, 2].astype(np.int64) - prefix[sent]).astype(np.float32)
    end = (idx[:, 3].astype(np.int64) - prefix[sent]).astype(np.float32)
    return np.stack([mlen, start, end], axis=1)  # [M, 3] f32


def _plan(mlen):
    """Sort mentions by descending sentence length and derive the shared
    per-(block, s-group) partition counts. Rank r -> core r%8, local row
    r//8. Core 0 holds the longest mention of every 8-rank stripe, so its
    profile upper-bounds all cores and one SPMD module is safe."""
    order = np.argsort(-mlen, kind="stable")
    lead = mlen[order[0::NCORES]]  # [ML], nonincreasing
    ks = tuple(
        tuple(
            int((lead[j * BLK : (j + 1) * BLK] > g * SG).sum()) for g in range(NG)
        )
        for j in range(NBLK)
    )
    return order, ks


def _prep_all(queries, values, idx, lengths):
    queries = np.asarray(queries, dtype=np.float32)
    values = np.asarray(values, dtype=np.float32)
    scal3 = _host_prep(idx, lengths)
    order, ks = _plan(scal3[:, 0])
    iota = np.ascontiguousarray(
        np.broadcast_to(np.arange(S, dtype=np.float32), (BLK, S))
    )
    in_maps = []
    for c in range(NCORES):
        sel = order[c::NCORES]
        in_maps.append(
            {
                "queries": np.ascontiguousarray(queries[sel]),
                "values": np.ascontiguousarray(values[sel]),
                "scal3": np.ascontiguousarray(scal3[sel]),
                "iota": iota,
            }
        )
    return order, ks, in_maps


def kernel(queries, values, idx, lengths):
    from concourse.bass_utils import run_bass_kernel_spmd

    order, ks, in_maps = _prep_all(queries, values, idx, lengths)
    nc = _build(1, ks)
    res = run_bass_kernel_spmd(nc, in_maps, core_ids=list(range(NCORES)))
    out = np.empty((M, S), dtype=np.float32)
    for c in range(NCORES):
        out[order[c::NCORES]] = res.results[c]["out"]
    return out
